# revision 43
# baseline (speedup 1.0000x reference)
"""Trainium2 Bass kernel for nn_DPSpikingDecoder.

Math: the leaky-integrator scan v_t = 0.5*v_{t-1} + x_t, the mean over
channels C, and the differential window pooling are all linear maps over
the time axis, so

    dp[b, w, f] = sum_{c,t} (K[w, t] / C) * spikes[b, c, t, f]

where K = M_pool @ L_scan is a [W=40, T=960] matrix.  K is exactly
Toeplitz across windows and K[w, t] = 0 for t >= 24(w+1), decaying as
0.5^(24w-t) into the past, so truncating it to a 2-window band (own
window + one window of history) changes dp by ~7e-9 relative.  The
stream is processed in t-major chunk order (chunk = 4 time steps x 32
channels = 128 rows); each chunk's matmul uses a 41-wide lhsT slice of
a zero-padded sliding band image whose only nonzero columns are the
chunk's own window and the next, so PSUM row w stops changing once
window w's own 6 chunks have streamed.  That lets the dpT extraction
(a batched selector matmul against 4 identity columns — partition-base
legal, one 128-col weight load per 4 windows) and the MLP layer-1
matmuls (bf16, single-pass) interleave into PE idle time during the
stream, pipelined a few groups behind the DMA, so after the last byte
only the last batch's finalize and the softmax tail remain instead of
the whole 80-matmul MLP.

The x stream reads HBM exactly once; with all 8 cores streaming their
sample concurrently the kernel runs at the shared per-core HBM
roofline (~350 GB/s).

Sharding: data-parallel over batch B=8 -> one sample per NeuronCore.
"""

import numpy as np
from contextlib import ExitStack

import concourse.bass as bass
import concourse.bacc as bacc
import concourse.tile as tile
from concourse import mybir
from concourse.bass_utils import run_bass_kernel_spmd

F32 = mybir.dt.float32
F32R = mybir.dt.float32r
BF16 = mybir.dt.bfloat16

B, C, T, F = 8, 32, 960, 256
L_DP, N_DP = 24, 12
W = T // L_DP            # 40 windows
H = 20                   # hidden dim of the MLP
CPG = 6                  # chunks per group (group == window: 24 t-steps)
TPG = L_DP               # t-steps per group
NCH = W * CPG            # 240 chunks total
KTW = 82                 # band image width: 40 zero cols + [j0, j1] + 40 zero
SB = 4                   # windows per dpT-extraction batch


def _host_K():
    """K[w, t] in float64: differential pooling of the decayed scan."""
    t = np.arange(T)
    d = t[:, None] - t[None, :]
    Lmat = np.where(d >= 0, 0.5 ** np.clip(d, 0, None), 0.0)
    M = np.zeros((W, T))
    for w in range(W):
        M[w, w * L_DP + L_DP - N_DP : w * L_DP + L_DP] = 1.0 / N_DP
        M[w, w * L_DP : w * L_DP + N_DP] -= 1.0 / N_DP
    return M @ Lmat  # [W, T]


def _host_ktw():
    """Sliding banded weight image [128, CPG*KTW]: chunk phase ph = 3*h + s
    covers t = 24*d + 12*h + 4*s + q for partition p = q*32 + c, and its
    weight for window w0+j is K[w0+j, 24*w0 + u], u = 12*(ph//3) +
    4*(ph%3) + p//32 — stationary in w0 (K is exactly Toeplitz).  The two
    live columns sit at [40, 42) so the lhsT slice [40-d : 81-d] puts them
    at out partitions d, d+1 while the PE output stays based at 0."""
    K = _host_K()
    w0 = 10  # any interior window
    ph = np.arange(CPG)[None, :]
    u = 12 * (ph // 3) + 4 * (ph % 3) + np.arange(128)[:, None] // 32  # [128, CPG]
    img = np.zeros((128, CPG, KTW))
    img[:, :, 40] = K[w0, 24 * w0 + u]
    img[:, :, 41] = K[w0 + 1, 24 * w0 + u]
    return np.ascontiguousarray((img / C).reshape(128, CPG * KTW).astype(np.float32))


def _host_cimg(W2, b2, b1):
    """Packed small consts [W+1, 122]: cols 0:40 rows 0:21 = [W2; b2] for
    the augmented layer-2 matmul, col 40 = b1, col 41 = ones (softmax-sum
    matmul), cols 42:82 = the identity selector (sel[w, d] = 1 iff w == d)
    for the dpT-extraction matmuls, cols 82:122 row 0 = a ones row for the
    rin-broadcast matmul."""
    img = np.zeros((W + 1, 122), dtype=np.float32)
    img[0:H, 0:W] = W2.astype(np.float32)
    img[H, 0:W] = b2.astype(np.float32)
    img[0:H, 40] = b1.astype(np.float32)
    img[:, 41] = 1.0
    img[0:W, 42:82] = np.eye(W, dtype=np.float32)
    img[0, 82:122] = 1.0
    return img


def _build_program():
    nc = bacc.Bacc(None)
    x = nc.declare_dram_parameter("x", [T, C, F], F32R, isOutput=False)
    ktw = nc.declare_dram_parameter("ktw", [128, CPG * KTW], F32R, isOutput=False)
    w1r = nc.declare_dram_parameter("w1r", [128, 2 * W * H], BF16, isOutput=False)
    cimg = nc.declare_dram_parameter("cimg", [W + 1, 122], F32, isOutput=False)
    y = nc.declare_dram_parameter("y", [W, F], F32, isOutput=True)

    with tile.TileContext(nc) as tc, ExitStack() as ctx:
        consts = ctx.enter_context(tc.tile_pool(name="consts", bufs=1))
        xs = ctx.enter_context(tc.tile_pool(name="xs", bufs=12))
        work = ctx.enter_context(tc.tile_pool(name="work", bufs=1))
        dp_psp = ctx.enter_context(tc.tile_pool(name="dp_ps", bufs=1, space="PSUM"))
        sm_ps = ctx.enter_context(tc.tile_pool(name="sm_ps", bufs=1, space="PSUM"))

        # small consts lead each queue; they land before the first stream
        # matmul / first window finalize needs them
        ktw_sb = consts.tile([128, CPG, KTW], F32R)
        nc.sync.dma_start(
            out=ktw_sb[:, 0:3, :],
            in_=ktw[:, 0 : 3 * KTW].rearrange("p (s k) -> p s k", k=KTW),
        )
        nc.scalar.dma_start(
            out=ktw_sb[:, 3:6, :],
            in_=ktw[:, 3 * KTW :].rearrange("p (s k) -> p s k", k=KTW),
        )
        ci_sb = consts.tile([W + 1, 122], F32)
        nc.scalar.dma_start(out=ci_sb, in_=cimg[:])
        w2b_sb = ci_sb[0 : H + 1, 0:W]
        b1_sb = ci_sb[0:H, 40:41]
        ones_col = ci_sb[0:W, 41:42]
        ones_row = ci_sb[0:1, 82:122]
        sel_sb = ci_sb[:, 42:82]
        w1_sb = consts.tile([128, 2 * W * H], BF16)

        # augmented MLP input [h; 1] so layer 2 adds b2 inside the matmul
        h_aug = work.tile([H + 1, 1], F32)
        nc.vector.memset(h_aug, 1.0)  # row H stays 1; rows 0..H-1 overwritten
        # row W is a dummy: the j=1 column of window 39's chunks lands there.
        # memset once so the selector matmuls never read uninitialized rows
        # (NaN * 0 = NaN).
        dp_sb = work.tile([W + 1, F], F32)
        nc.vector.memset(dp_sb, 0.0)
        dpT_sb = work.tile([128, 2, 2, SB], BF16)  # [f, batch parity, e, slot]
        att = work.tile([W, F], F32)

        dp_ps = dp_psp.tile([W + 1, F], F32)
        dpT_ps = sm_ps.tile([128, 2, 2, SB], F32)
        hp_ps = sm_ps.tile([H, 1], F32)

        def sel_batch(w0):
            """dpT slots <- dp rows w0..w0+SB-1 via dp_sb^T @ sel[:, w0:w0+SB]:
            one 128-col weight load per e-half covers SB windows."""
            par = (w0 // SB) % 2
            for e in range(2):
                nc.tensor.matmul(
                    dpT_ps[:, par, e, :],
                    lhsT=dp_sb[:, e * 128 : (e + 1) * 128],
                    rhs=sel_sb[:, w0 : w0 + SB],
                    start=True,
                    stop=True,
                )
            nc.vector.tensor_copy(dpT_sb[:, par], dpT_ps[:, par])

        def l1_mms(w):
            """hp += W1_w^T @ dpT_w for both f-halves (bf16: single-pass,
            the rounding error is far inside the output tolerance)."""
            par = (w // SB) % 2
            for e in range(2):
                mi = 2 * w + e
                nc.tensor.matmul(
                    hp_ps,
                    lhsT=w1_sb[:, mi * H : (mi + 1) * H],
                    rhs=dpT_sb[:, par, e, w % SB : w % SB + 1],
                    start=(mi == 0),
                    stop=(mi == 2 * W - 1),
                )

        for d in range(W):
            if d == 3:
                # w1 (first needed by l1_mms at group 5) rides behind the
                # first three x groups so it does not delay the stream start
                nc.sync.dma_start(out=w1_sb[:, 0 : W * H], in_=w1r[:, 0 : W * H])
                nc.scalar.dma_start(out=w1_sb[:, W * H :], in_=w1r[:, W * H :])
            xt = xs.tile([128, CPG, F], F32R)
            for eng, h2 in ((nc.sync, 0), (nc.scalar, 1)):
                t0 = TPG * d + 12 * h2
                eng.dma_start(
                    out=xt[:, 3 * h2 : 3 * h2 + 3, :],
                    in_=x[t0 : t0 + 12, :, :].rearrange("(s q) c f -> (q c) s f", q=4),
                )
            for s in range(CPG):
                m = d * CPG + s
                nc.tensor.matmul(
                    dp_ps,
                    lhsT=ktw_sb[:, s, 40 - d : 81 - d],
                    rhs=xt[:, s, :],
                    start=(m == 0),
                    stop=(m == NCH - 1),
                )
            # windows finalize progressively.  dp rows are copied out of
            # PSUM only at batch boundaries (a per-group copy would put a
            # PE->DVE->PE semaphore chain between every pair of groups);
            # the copy spans [0, d) because engine PSUM reads must start at
            # an aligned partition — same DVE latency, partitions are lanes.
            if d % SB == 0 and d >= SB:
                nc.vector.tensor_copy(dp_sb[0:d, :], dp_ps[0:d, :])
                sel_batch(d - SB)
            if d >= SB + 1:
                l1_mms(d - SB - 1)

        # ---- drain the finalize pipeline: windows 35..39 ----
        l1_mms(W - SB - 1)
        nc.vector.tensor_copy(dp_sb, dp_ps)
        sel_batch(W - SB)
        for w in range(W - SB, W):
            l1_mms(w)

        # ---- tail: relu -> layer 2 (+b2) -> softmax -> scale -> store,
        # all column-oriented so the scale is one fused two-scalar DVE op
        nc.scalar.activation(
            h_aug[0:H, :], hp_ps, mybir.ActivationFunctionType.Relu, bias=b1_sb
        )
        a2c_ps = sm_ps.tile([W, 1], F32)
        nc.tensor.matmul(a2c_ps, lhsT=w2b_sb, rhs=h_aug[:], start=True, stop=True)
        e_col = work.tile([W, 1], F32)
        nc.scalar.activation(e_col, a2c_ps, mybir.ActivationFunctionType.Exp)
        ssum_ps = sm_ps.tile([1, 1], F32)
        nc.tensor.matmul(ssum_ps, lhsT=e_col[:], rhs=ones_col, start=True, stop=True)
        rin = work.tile([1, 1], F32)
        nc.vector.reciprocal(rin, ssum_ps)
        rb_ps = sm_ps.tile([W, 1], F32)
        nc.tensor.matmul(rb_ps, lhsT=ones_row, rhs=rin[:], start=True, stop=True)
        # fused (dp * e) * (1/sum); the broadcast 1/sum column is read
        # straight from PSUM by the DVE
        nc.vector.tensor_scalar(
            att,
            dp_sb[0:W, :],
            e_col[:],
            rb_ps[:],
            mybir.AluOpType.mult,
            mybir.AluOpType.mult,
        )
        nc.sync.dma_start(out=y[:, 0:128], in_=att[:, 0:128])
        nc.scalar.dma_start(out=y[:, 128:256], in_=att[:, 128:256])

    nc.compile()
    return nc


_CACHED = {}


def _get_program():
    if "nc" not in _CACHED:
        _CACHED["nc"] = _build_program()
        _CACHED["ktw"] = _host_ktw()
    return _CACHED["nc"]


def _in_maps(spikes, W1, b1, W2, b2):
    spikes = np.ascontiguousarray(np.asarray(spikes, dtype=np.float32))
    W1 = np.asarray(W1, dtype=np.float32)
    b1 = np.asarray(b1, dtype=np.float32)
    W2 = np.asarray(W2, dtype=np.float32)
    b2 = np.asarray(b2, dtype=np.float32)
    _get_program()
    # W1 rearranged so chunk mi = 2*w + e holds rows d = 256*w + 128*e + p,
    # laid out so the DMA is one contiguous [128, 1600] block.
    import ml_dtypes

    w1r = np.ascontiguousarray(
        W1.reshape(W, 2, 128, H)
        .transpose(2, 0, 1, 3)
        .reshape(128, 2 * W * H)
        .astype(ml_dtypes.bfloat16)
    )
    cimg = _host_cimg(W2, b2, b1)
    shared = {"ktw": _CACHED["ktw"], "w1r": w1r, "cimg": cimg}
    # t-major layout [T, C, F] so each DMA descriptor's source is a long
    # contiguous run (c-adjacent rows) and the partition grouping is clean
    return [
        {"x": np.ascontiguousarray(spikes[b].transpose(1, 0, 2)), **shared}
        for b in range(B)
    ]


def kernel(spikes, W1, b1, W2, b2):
    in_maps = _in_maps(spikes, W1, b1, W2, b2)
    res = run_bass_kernel_spmd(_get_program(), in_maps, list(range(B)))
    out = np.stack([np.asarray(res.results[i]["y"]).reshape(W * F) for i in range(B)])
    return out.astype(np.float32)


# revision 44
# speedup vs baseline: 1.0106x; 1.0106x over previous
"""Trainium2 Bass kernel for nn_DPSpikingDecoder.

Math: the leaky-integrator scan v_t = 0.5*v_{t-1} + x_t, the mean over
channels C, and the differential window pooling are all linear maps over
the time axis, so

    dp[b, w, f] = sum_{c,t} (K[w, t] / C) * spikes[b, c, t, f]

where K = M_pool @ L_scan is a [W=40, T=960] matrix.  K is exactly
Toeplitz across windows and K[w, t] = 0 for t >= 24(w+1), decaying as
0.5^(24w-t) into the past, so truncating it to a 2-window band (own
window + one window of history) changes dp by ~7e-9 relative.  The
stream is processed in t-major chunk order (chunk = 4 time steps x 32
channels = 128 rows); each chunk's matmul uses a 41-wide lhsT slice of
a zero-padded sliding band image whose only nonzero columns are the
chunk's own window and the next, so PSUM row w stops changing once
window w's own 6 chunks have streamed.  That lets the dpT extraction
(a batched selector matmul against 4 identity columns — partition-base
legal, one 128-col weight load per 4 windows) and the MLP layer-1
matmuls (bf16, single-pass) interleave into PE idle time during the
stream, pipelined a few groups behind the DMA, so after the last byte
only the last batch's finalize and the softmax tail remain instead of
the whole 80-matmul MLP.

The x stream reads HBM exactly once; with all 8 cores streaming their
sample concurrently the kernel runs at the shared per-core HBM
roofline (~350 GB/s).

Sharding: data-parallel over batch B=8 -> one sample per NeuronCore.
"""

import numpy as np
from contextlib import ExitStack

import concourse.bass as bass
import concourse.bacc as bacc
import concourse.tile as tile
from concourse import mybir
from concourse.bass_utils import run_bass_kernel_spmd

F32 = mybir.dt.float32
F32R = mybir.dt.float32r
BF16 = mybir.dt.bfloat16

B, C, T, F = 8, 32, 960, 256
L_DP, N_DP = 24, 12
W = T // L_DP            # 40 windows
H = 20                   # hidden dim of the MLP
CPG = 6                  # chunks per group (group == window: 24 t-steps)
TPG = L_DP               # t-steps per group
NCH = W * CPG            # 240 chunks total
KTW = 82                 # band image width: 40 zero cols + [j0, j1] + 40 zero
SB = 4                   # windows per dpT-extraction batch


def _host_K():
    """K[w, t] in float64: differential pooling of the decayed scan."""
    t = np.arange(T)
    d = t[:, None] - t[None, :]
    Lmat = np.where(d >= 0, 0.5 ** np.clip(d, 0, None), 0.0)
    M = np.zeros((W, T))
    for w in range(W):
        M[w, w * L_DP + L_DP - N_DP : w * L_DP + L_DP] = 1.0 / N_DP
        M[w, w * L_DP : w * L_DP + N_DP] -= 1.0 / N_DP
    return M @ Lmat  # [W, T]


def _host_ktw():
    """Sliding banded weight image [128, CPG*KTW]: chunk phase ph = 3*h + s
    covers t = 24*d + 12*h + 4*s + q for partition p = q*32 + c, and its
    weight for window w0+j is K[w0+j, 24*w0 + u], u = 12*(ph//3) +
    4*(ph%3) + p//32 — stationary in w0 (K is exactly Toeplitz).  The two
    live columns sit at [40, 42) so the lhsT slice [40-d : 81-d] puts them
    at out partitions d, d+1 while the PE output stays based at 0."""
    K = _host_K()
    w0 = 10  # any interior window
    ph = np.arange(CPG)[None, :]
    u = 12 * (ph // 3) + 4 * (ph % 3) + np.arange(128)[:, None] // 32  # [128, CPG]
    img = np.zeros((128, CPG, KTW))
    img[:, :, 40] = K[w0, 24 * w0 + u]
    img[:, :, 41] = K[w0 + 1, 24 * w0 + u]
    return np.ascontiguousarray((img / C).reshape(128, CPG * KTW).astype(np.float32))


def _host_cimg(W2, b2, b1):
    """Packed small consts [W+1, 122]: cols 0:40 rows 0:21 = [W2; b2] for
    the augmented layer-2 matmul, col 40 = b1, col 41 = ones (softmax-sum
    matmul), cols 42:82 = the identity selector (sel[w, d] = 1 iff w == d)
    for the dpT-extraction matmuls, cols 82:122 row 0 = a ones row for the
    rin-broadcast matmul."""
    img = np.zeros((W + 1, 122), dtype=np.float32)
    img[0:H, 0:W] = W2.astype(np.float32)
    img[H, 0:W] = b2.astype(np.float32)
    img[0:H, 40] = b1.astype(np.float32)
    img[:, 41] = 1.0
    img[0:W, 42:82] = np.eye(W, dtype=np.float32)
    img[0, 82:122] = 1.0
    return img


def _build_program():
    nc = bacc.Bacc(None)
    x = nc.declare_dram_parameter("x", [T, C, F], F32R, isOutput=False)
    ktw = nc.declare_dram_parameter("ktw", [128, CPG * KTW], F32R, isOutput=False)
    w1r = nc.declare_dram_parameter("w1r", [128, 2 * W * H], BF16, isOutput=False)
    cimg = nc.declare_dram_parameter("cimg", [W + 1, 122], F32, isOutput=False)
    y = nc.declare_dram_parameter("y", [W, F], F32, isOutput=True)

    with tile.TileContext(nc) as tc, ExitStack() as ctx:
        consts = ctx.enter_context(tc.tile_pool(name="consts", bufs=1))
        xs = ctx.enter_context(tc.tile_pool(name="xs", bufs=12))
        work = ctx.enter_context(tc.tile_pool(name="work", bufs=1))
        dp_psp = ctx.enter_context(tc.tile_pool(name="dp_ps", bufs=1, space="PSUM"))
        sm_ps = ctx.enter_context(tc.tile_pool(name="sm_ps", bufs=1, space="PSUM"))

        # small consts lead each queue; they land before the first stream
        # matmul / first window finalize needs them
        ktw_sb = consts.tile([128, CPG, KTW], F32R)
        nc.sync.dma_start(out=ktw_sb, in_=ktw[:].rearrange("p (s k) -> p s k", k=KTW))
        ci_sb = consts.tile([W + 1, 122], F32)
        nc.scalar.dma_start(out=ci_sb, in_=cimg[:])
        w2b_sb = ci_sb[0 : H + 1, 0:W]
        b1_sb = ci_sb[0:H, 40:41]
        ones_col = ci_sb[0:W, 41:42]
        ones_row = ci_sb[0:1, 82:122]
        sel_sb = ci_sb[:, 42:82]
        w1_sb = consts.tile([128, 2 * W * H], BF16)

        # augmented MLP input [h; 1] so layer 2 adds b2 inside the matmul
        h_aug = work.tile([H + 1, 1], F32)
        nc.vector.memset(h_aug, 1.0)  # row H stays 1; rows 0..H-1 overwritten
        # row W is a dummy: the j=1 column of window 39's chunks lands there.
        # memset once so the selector matmuls never read uninitialized rows
        # (NaN * 0 = NaN).
        dp_sb = work.tile([W + 1, F], F32)
        nc.vector.memset(dp_sb, 0.0)
        dpT_sb = work.tile([128, 2, 2, SB], BF16)  # [f, batch parity, e, slot]
        att = work.tile([W, F], F32)

        dp_ps = dp_psp.tile([W + 1, F], F32)
        dpT_ps = sm_ps.tile([128, 2, 2, SB], F32)
        hp_ps = sm_ps.tile([H, 1], F32)

        def sel_batch(w0):
            """dpT slots <- dp rows w0..w0+SB-1 via dp_sb^T @ sel[:, w0:w0+SB]:
            one 128-col weight load per e-half covers SB windows."""
            par = (w0 // SB) % 2
            for e in range(2):
                nc.tensor.matmul(
                    dpT_ps[:, par, e, :],
                    lhsT=dp_sb[:, e * 128 : (e + 1) * 128],
                    rhs=sel_sb[:, w0 : w0 + SB],
                    start=True,
                    stop=True,
                )
            nc.vector.tensor_copy(dpT_sb[:, par], dpT_ps[:, par])

        def l1_mms(w):
            """hp += W1_w^T @ dpT_w for both f-halves (bf16: single-pass,
            the rounding error is far inside the output tolerance)."""
            par = (w // SB) % 2
            for e in range(2):
                mi = 2 * w + e
                nc.tensor.matmul(
                    hp_ps,
                    lhsT=w1_sb[:, mi * H : (mi + 1) * H],
                    rhs=dpT_sb[:, par, e, w % SB : w % SB + 1],
                    start=(mi == 0),
                    stop=(mi == 2 * W - 1),
                )

        for d in range(W):
            if d == 3:
                # w1 (first needed by l1_mms at group 5) rides behind the
                # first three x groups so it does not delay the stream start
                nc.sync.dma_start(out=w1_sb[:, 0 : W * H], in_=w1r[:, 0 : W * H])
                nc.scalar.dma_start(out=w1_sb[:, W * H :], in_=w1r[:, W * H :])
            xt = xs.tile([128, CPG, F], F32R)
            for eng, h2 in ((nc.sync, 0), (nc.scalar, 1)):
                t0 = TPG * d + 12 * h2
                eng.dma_start(
                    out=xt[:, 3 * h2 : 3 * h2 + 3, :],
                    in_=x[t0 : t0 + 12, :, :].rearrange("(s q) c f -> (q c) s f", q=4),
                )
            for s in range(CPG):
                m = d * CPG + s
                nc.tensor.matmul(
                    dp_ps,
                    lhsT=ktw_sb[:, s, 40 - d : 81 - d],
                    rhs=xt[:, s, :],
                    start=(m == 0),
                    stop=(m == NCH - 1),
                )
            # windows finalize progressively.  dp rows are copied out of
            # PSUM only at batch boundaries (a per-group copy would put a
            # PE->DVE->PE semaphore chain between every pair of groups);
            # the copy spans [0, d) because engine PSUM reads must start at
            # an aligned partition — same DVE latency, partitions are lanes.
            if d % SB == 0 and d >= SB:
                nc.vector.tensor_copy(dp_sb[0:d, :], dp_ps[0:d, :])
                sel_batch(d - SB)
            if d >= SB + 1:
                l1_mms(d - SB - 1)

        # ---- drain the finalize pipeline: windows 35..39 ----
        l1_mms(W - SB - 1)
        nc.vector.tensor_copy(dp_sb, dp_ps)
        sel_batch(W - SB)
        for w in range(W - SB, W):
            l1_mms(w)

        # ---- tail: relu -> layer 2 (+b2) -> softmax -> scale -> store,
        # all column-oriented so the scale is one fused two-scalar DVE op
        nc.scalar.activation(
            h_aug[0:H, :], hp_ps, mybir.ActivationFunctionType.Relu, bias=b1_sb
        )
        a2c_ps = sm_ps.tile([W, 1], F32)
        nc.tensor.matmul(a2c_ps, lhsT=w2b_sb, rhs=h_aug[:], start=True, stop=True)
        e_col = work.tile([W, 1], F32)
        nc.scalar.activation(e_col, a2c_ps, mybir.ActivationFunctionType.Exp)
        ssum_ps = sm_ps.tile([1, 1], F32)
        nc.tensor.matmul(ssum_ps, lhsT=e_col[:], rhs=ones_col, start=True, stop=True)
        rin = work.tile([1, 1], F32)
        nc.vector.reciprocal(rin, ssum_ps)
        rb_ps = sm_ps.tile([W, 1], F32)
        nc.tensor.matmul(rb_ps, lhsT=ones_row, rhs=rin[:], start=True, stop=True)
        # fused (dp * e) * (1/sum) per half, storing each as soon as it is
        # scaled; the broadcast 1/sum column is read straight from PSUM
        for eng, e2 in ((nc.sync, 0), (nc.scalar, 1)):
            nc.vector.tensor_scalar(
                att[:, e2 * 128 : (e2 + 1) * 128],
                dp_sb[0:W, e2 * 128 : (e2 + 1) * 128],
                e_col[:],
                rb_ps[:],
                mybir.AluOpType.mult,
                mybir.AluOpType.mult,
            )
            eng.dma_start(
                out=y[:, e2 * 128 : (e2 + 1) * 128],
                in_=att[:, e2 * 128 : (e2 + 1) * 128],
            )

    nc.compile()
    return nc


_CACHED = {}


def _get_program():
    if "nc" not in _CACHED:
        _CACHED["nc"] = _build_program()
        _CACHED["ktw"] = _host_ktw()
    return _CACHED["nc"]


def _in_maps(spikes, W1, b1, W2, b2):
    spikes = np.ascontiguousarray(np.asarray(spikes, dtype=np.float32))
    W1 = np.asarray(W1, dtype=np.float32)
    b1 = np.asarray(b1, dtype=np.float32)
    W2 = np.asarray(W2, dtype=np.float32)
    b2 = np.asarray(b2, dtype=np.float32)
    _get_program()
    # W1 rearranged so chunk mi = 2*w + e holds rows d = 256*w + 128*e + p,
    # laid out so the DMA is one contiguous [128, 1600] block.
    import ml_dtypes

    w1r = np.ascontiguousarray(
        W1.reshape(W, 2, 128, H)
        .transpose(2, 0, 1, 3)
        .reshape(128, 2 * W * H)
        .astype(ml_dtypes.bfloat16)
    )
    cimg = _host_cimg(W2, b2, b1)
    shared = {"ktw": _CACHED["ktw"], "w1r": w1r, "cimg": cimg}
    # t-major layout [T, C, F] so each DMA descriptor's source is a long
    # contiguous run (c-adjacent rows) and the partition grouping is clean
    return [
        {"x": np.ascontiguousarray(spikes[b].transpose(1, 0, 2)), **shared}
        for b in range(B)
    ]


def kernel(spikes, W1, b1, W2, b2):
    in_maps = _in_maps(spikes, W1, b1, W2, b2)
    res = run_bass_kernel_spmd(_get_program(), in_maps, list(range(B)))
    out = np.stack([np.asarray(res.results[i]["y"]).reshape(W * F) for i in range(B)])
    return out.astype(np.float32)
